# revision 4
# baseline (speedup 1.0000x reference)
"""Trainium2 Bass kernel for nn_Message (GNN message passing), 8 NeuronCores.

Math (from the reference):
    q = (rbf @ Wf + bf) * cc * h            per node, [N, 384]
      rbf[n,k] = sin((k+1)*pi*d_n/5)/d_n, cc = cosine cutoff(d),
      h = silu(node_s @ W1 + b1) @ W2 + b2
    payload m[n] = [ q[:,256:384] (128) | (node_vec * q[:,0:128])(384) ]
    out[d] = base[d] + sum_{edges e: dst[e]=d} m[src[e]],  base = [node_s | node_vec]

Distribution (v2):
  - Nodes in 6400-blocks per core.  Phase 1 computes the bf16 payload slice
    node-major (no output transposes): h1T -> silu -> h = aT.T @ W2cat and
    F2 = sinT.T @ Wfcat_aug; sin(k*theta)*cc/d via Chebyshev recurrence,
    bf*cc folded in as a 21st rbf row; b2 pre-broadcast from host.
  - AllGather in 4 chunks (tile spans) overlapping phase-1; table stored
    chunk-major, host remaps src indices.  Rows [0,25600) ready after AG1.
  - Phase 2 in two passes: LO (table rows < 25600) then HI, each a
    continuous stream of uniform 1024-idx dma_gathers (the SWDGE ring cap)
    crossing window boundaries; one-hot is_equal matmuls accumulate per
    128-dst window in PSUM; LO results parked in a bf16 SBUF accumulator.
"""
import sys, os
sys.path.insert(0, "/opt/trn_rl_repo")
import numpy as np
import ml_dtypes

BF16 = ml_dtypes.bfloat16

N, F, E = 50000, 128, 800000
CUTOFF = 5.0
NUM_RBF = 20
K1 = NUM_RBF + 1
NC = 8
SLICE = 6400
NPAD = SLICE * NC             # 51200
HALF = NPAD // 2              # 25600
NT = SLICE // 128             # 50 windows / node tiles per core
D = 512

# Two AllGather chunks: lo = first 3200 nodes of each core slice, hi = rest.
HSL = SLICE // 2  # 3200


def _remap(n):
    """Global node id -> (row in its half-table, is_lo).  Each half-table is
    [NC*3200, D]: rank-major concat of per-core half-slices."""
    n = np.asarray(n, np.int64)
    k = n % SLICE
    r = n // SLICE
    lo = k < HSL
    row = r * HSL + np.where(lo, k, k - HSL)
    return row, lo


def _pack_idx(idx, ntok):
    a = np.asarray(idx, np.int16).reshape(ntok // 16, 16).T
    return np.tile(a, (8, 1))


def _host_prep(node_s, node_vec, edge, edge_dis):
    dst = edge[:, 0].astype(np.int64)
    src = edge[:, 1].astype(np.int64)
    srow, slo = _remap(src)

    order = np.argsort(dst, kind="stable")
    dst_s = dst[order]
    srow_s = srow[order]
    core_of = dst_s // SLICE
    win_of = (dst_s % SLICE) // 128
    loc_of = dst_s % 128
    lo_mask = slo[order]

    gwin = core_of * NT + win_of
    bounds = np.searchsorted(gwin, np.arange(NC * NT + 1))

    counts_lo = np.zeros((NC, NT), np.int64)
    counts_hi = np.zeros((NC, NT), np.int64)
    per_win = []
    for g in range(NC * NT):
        s, e_ = bounds[g], bounds[g + 1]
        m = lo_mask[s:e_]
        per_win.append((srow_s[s:e_][m], loc_of[s:e_][m],
                        srow_s[s:e_][~m], loc_of[s:e_][~m]))
        c, w = divmod(g, NT)
        counts_lo[c, w] = int(m.sum())
        counts_hi[c, w] = int((~m).sum())

    TA = np.maximum(1, -(-counts_lo.max(0) // 128)).astype(int)   # tiles per window, lo
    TB = np.maximum(1, -(-counts_hi.max(0) // 128)).astype(int)
    offLO = np.concatenate([[0], np.cumsum(TA)])
    offHI = np.concatenate([[0], np.cumsum(TB)])
    TLO, THI = int(offLO[-1]), int(offHI[-1])

    ns_pad = np.zeros((NPAD, F), np.float32)
    ns_pad[:N] = node_s
    nv_pad = np.zeros((NPAD, F * 3), np.float32)
    nv_pad[:N] = node_vec.reshape(N, F * 3)
    d_pad = np.ones(NPAD, np.float32)
    d_pad[:N] = edge_dis
    base_all = np.concatenate([ns_pad, nv_pad], 1)

    in_maps = []
    for c in range(NC):
        idx_lo = np.zeros((128, TLO * 8), np.int16)
        idx_hi = np.zeros((128, THI * 8), np.int16)
        dst_lo = np.full((128, TLO), -1.0, BF16)
        dst_hi = np.full((128, THI), -1.0, BF16)
        for w in range(NT):
            ls, ll, hs, hl = per_win[c * NT + w]
            nA, nB = TA[w] * 128, TB[w] * 128
            la = np.zeros(nA, np.int64); la[: len(ls)] = ls
            idx_lo[:, offLO[w] * 8 : offLO[w + 1] * 8] = _pack_idx(la, nA)
            hb = np.zeros(nB, np.int64); hb[: len(hs)] = hs
            idx_hi[:, offHI[w] * 8 : offHI[w + 1] * 8] = _pack_idx(hb, nB)
            dl = np.full(nA, -1.0, np.float32); dl[: len(ll)] = ll
            dst_lo[:, offLO[w] : offLO[w + 1]] = dl.reshape(TA[w], 128).T.astype(BF16)
            dh = np.full(nB, -1.0, np.float32); dh[: len(hl)] = hl
            dst_hi[:, offHI[w] : offHI[w + 1]] = dh.reshape(TB[w], 128).T.astype(BF16)
        sl = slice(c * SLICE, (c + 1) * SLICE)
        in_maps.append({
            "base": base_all[sl].copy(),
            "nsT": ns_pad[sl].T.astype(BF16).copy(),          # [128, 6400]
            "d_pt": d_pad[sl].reshape(NT, 128).T.copy(),      # [128, NT]
            "idx_lo": idx_lo, "idx_hi": idx_hi,
            "dst_lo": dst_lo, "dst_hi": dst_hi,
        })
    return in_maps, TA, TB, offLO, offHI, TLO, THI


def _build_program(TA, TB, offLO, offHI, TLO, THI):
    import concourse.bass as bass
    import concourse.bacc as bacc
    import concourse.mybir as mybir
    import concourse.tile as tile
    from concourse import library_config
    from concourse.masks import make_identity

    FP = mybir.dt.float32
    BF = mybir.dt.bfloat16
    AF = mybir.ActivationFunctionType
    OP = mybir.AluOpType

    nc = bacc.Bacc("TRN2", target_bir_lowering=False, debug=False, num_devices=NC)

    base_h = nc.dram_tensor("base", [SLICE, D], FP, kind="ExternalInput")
    nsT_h = nc.dram_tensor("nsT", [128, SLICE], BF, kind="ExternalInput")
    dpt_h = nc.dram_tensor("d_pt", [128, NT], FP, kind="ExternalInput")
    ilo_h = nc.dram_tensor("idx_lo", [128, TLO * 8], mybir.dt.int16, kind="ExternalInput")
    ihi_h = nc.dram_tensor("idx_hi", [128, THI * 8], mybir.dt.int16, kind="ExternalInput")
    dlo_h = nc.dram_tensor("dst_lo", [128, TLO], BF, kind="ExternalInput")
    dhi_h = nc.dram_tensor("dst_hi", [128, THI], BF, kind="ExternalInput")
    W1_h = nc.dram_tensor("W1b", [F, F], BF, kind="ExternalInput")
    W2c_h = nc.dram_tensor("W2c", [F, 256], BF, kind="ExternalInput")
    Wfc_h = nc.dram_tensor("Wfc", [K1, 256], BF, kind="ExternalInput")
    b1_h = nc.dram_tensor("b1c", [F, 1], FP, kind="ExternalInput")
    b2bc_h = nc.dram_tensor("b2bc", [128, 256], FP, kind="ExternalInput")
    iota_h = nc.dram_tensor("iota", [128, 128], BF, kind="ExternalInput")
    out_h = nc.dram_tensor("out", [SLICE, D], FP, kind="ExternalOutput")

    with tile.TileContext(nc) as tc:
        nc.gpsimd.load_library(library_config.mlp)
        with (
            tc.tile_pool(name="dram", bufs=1, space="DRAM") as dram,
            tc.tile_pool(name="const", bufs=1) as cp,
            tc.tile_pool(name="work", bufs=2) as wp,
            tc.tile_pool(name="prep", bufs=1) as pp,
            tc.tile_pool(name="ps1", bufs=1, space="PSUM") as ps1,
            tc.tile_pool(name="ps2", bufs=3, space="PSUM") as ps2,
        ):
            # ---- constants ----
            identb = cp.tile([128, 128], BF)
            make_identity(nc, identb[:])
            W1s = cp.tile([F, F], BF); nc.sync.dma_start(W1s[:], W1_h[:])
            W2c = cp.tile([F, 256], BF); nc.sync.dma_start(W2c[:], W2c_h[:])
            Wfc = cp.tile([K1, 256], BF); nc.sync.dma_start(Wfc[:], Wfc_h[:])
            b1s = cp.tile([F, 1], FP); nc.sync.dma_start(b1s[:], b1_h[:])
            b2bc = cp.tile([128, 256], FP); nc.sync.dma_start(b2bc[:], b2bc_h[:])
            iota = cp.tile([128, 128], BF); nc.sync.dma_start(iota[:], iota_h[:])
            halfpi = cp.tile([128, 1], FP)
            nc.vector.memset(halfpi[:], float(np.pi / 2))
            nsT = cp.tile([128, SLICE], BF)
            nc.sync.dma_start(nsT[:], nsT_h[:])

            m_slice = dram.tile([SLICE, D], BF)
            m_lo = dram.tile([NC * HSL, D], BF, addr_space="Shared")
            m_hi = dram.tile([NC * HSL, D], BF, addr_space="Shared")

            # ---- prepass: srec[:,k,t] = sin((k+1)theta)*cc/d, srec[:,20,:]=cc ----
            d_all = pp.tile([128, NT], FP)
            nc.sync.dma_start(d_all[:], dpt_h[:])
            srec = pp.tile([128, K1, NT], FP)
            sh = pp.tile([128, NT], FP)
            ch = pp.tile([128, NT], FP)
            nc.scalar.activation(sh[:], d_all[:], AF.Sin, scale=float(np.pi / 10))
            nc.scalar.activation(ch[:], d_all[:], AF.Sin, bias=halfpi[:], scale=float(-np.pi / 10))
            s1 = srec[:, 0, :]
            nc.vector.tensor_mul(s1, sh[:], ch[:])
            nc.vector.tensor_scalar_mul(s1, s1, 2.0)
            cth = pp.tile([128, NT], FP)
            nc.vector.tensor_mul(cth[:], sh[:], sh[:])
            nc.vector.tensor_scalar(cth[:], cth[:], -2.0, 1.0, OP.mult, OP.add)
            c2 = pp.tile([128, NT], FP)
            nc.vector.tensor_scalar_mul(c2[:], cth[:], 2.0)
            nc.vector.tensor_mul(srec[:, 1, :], c2[:], s1)
            for k in range(2, NUM_RBF):
                nc.vector.tensor_mul(srec[:, k, :], c2[:], srec[:, k - 1, :])
                nc.vector.tensor_sub(srec[:, k, :], srec[:, k, :], srec[:, k - 2, :])
            mask = pp.tile([128, NT], FP)
            nc.vector.tensor_scalar(mask[:], d_all[:], CUTOFF, None, OP.is_le)
            cc = srec[:, NUM_RBF, :]
            nc.vector.tensor_scalar(cc, cth[:], 0.5, 0.5, OP.mult, OP.add)
            nc.vector.tensor_mul(cc, cc, mask[:])
            sall = pp.tile([128, NT], FP)
            nc.vector.reciprocal(sall[:], d_all[:])
            nc.vector.tensor_mul(sall[:], sall[:], cc)
            nc.vector.tensor_tensor(
                out=srec[:, 0:NUM_RBF, :], in0=srec[:, 0:NUM_RBF, :],
                in1=sall[:, None, :].to_broadcast([128, NUM_RBF, NT]), op=OP.mult)
            srecb = pp.tile([128, K1, NT], BF)
            nc.vector.tensor_copy(srecb[:], srec[:])

            # ---- phase 1 + two AGs ----
            for j, (t0j, t1j) in enumerate(((0, NT // 2), (NT // 2, NT))):
                for t in range(t0j, t1j):
                    r0, r1 = t * 128, (t + 1) * 128
                    h1_ps = ps1.tile([128, F], FP, tag="h1", space="PSUM", bufs=2)
                    nc.tensor.matmul(out=h1_ps[:], lhsT=W1s[:], rhs=nsT[:, r0:r1],
                                     start=True, stop=True)
                    aT = wp.tile([128, F], BF, tag="aT")
                    nc.scalar.activation(aT[:], h1_ps[:], AF.Silu, bias=b1s[:])
                    sinT_ps = ps1.tile([K1, 128], BF, tag="st", space="PSUM")
                    nc.tensor.transpose(out=sinT_ps[:], in_=srecb[:, :, t], identity=identb[:])
                    sinT = wp.tile([K1, 128], BF, tag="stb")
                    nc.vector.tensor_copy(out=sinT[:], in_=sinT_ps[:])
                    h_ps = ps1.tile([128, 256], FP, tag="h2", space="PSUM")
                    nc.tensor.matmul(out=h_ps[:], lhsT=aT[:], rhs=W2c[:], start=True, stop=True)
                    F2_ps = ps1.tile([128, 256], FP, tag="f2", space="PSUM")
                    nc.tensor.matmul(out=F2_ps[:], lhsT=sinT[:], rhs=Wfc[:], start=True, stop=True)
                    hb = wp.tile([128, 256], FP, tag="hb")
                    nc.vector.tensor_add(hb[:], h_ps[:], b2bc[:])
                    q = wp.tile([128, 256], FP, tag="q")
                    nc.vector.tensor_mul(q[:], F2_ps[:], hb[:])
                    pay = wp.tile([128, D], BF, tag="pay")
                    nc.vector.tensor_copy(pay[:, 0:128], q[:, 128:256])
                    NV = wp.tile([128, F * 3], FP, tag="NV")
                    nc.sync.dma_start(NV[:], base_h[r0:r1, 128:512])
                    nc.vector.tensor_tensor(
                        out=pay[:, 128:512].rearrange("p (f c) -> p f c", c=3),
                        in0=NV[:].rearrange("p (f c) -> p f c", c=3),
                        in1=q[:, 0:128, None].to_broadcast([128, F, 3]),
                        op=OP.mult)
                    nc.sync.dma_start(m_slice[r0:r1, :], pay[:])
                nc.gpsimd.collective_compute(
                    "AllGather", OP.bypass,
                    replica_groups=[list(range(NC))],
                    ins=[m_slice[j * HSL : (j + 1) * HSL, :].opt()],
                    outs=[(m_lo if j == 0 else m_hi)[:].opt()],
                )

            # ---- phase 2 ----
            ilo = pp.tile([128, TLO * 8], mybir.dt.int16)
            nc.sync.dma_start(ilo[:], ilo_h[:])
            ihi = pp.tile([128, THI * 8], mybir.dt.int16)
            nc.sync.dma_start(ihi[:], ihi_h[:])
            dlo = pp.tile([128, TLO], BF)
            nc.sync.dma_start(dlo[:], dlo_h[:])
            dhi = pp.tile([128, THI], BF)
            nc.sync.dma_start(dhi[:], dhi_h[:])
            acc16 = pp.tile([128, NT * D], BF)

            for hi_pass in (False, True):
                ntiles = THI if hi_pass else TLO
                idx_t = ihi if hi_pass else ilo
                dst_t = dhi if hi_pass else dlo
                tab = (m_hi if hi_pass else m_lo)[:]
                off = offHI if hi_pass else offLO
                TT = TB if hi_pass else TA
                nchunks = -(-ntiles // 8)
                gtiles = []
                for c in range(nchunks):
                    cn = min(8, ntiles - c * 8)
                    g = wp.tile([128, 8, D], BF, tag="g", bufs=4)
                    nc.gpsimd.dma_gather(
                        g[:, 0:cn, :], tab, idx_t[:, c * 64 : c * 64 + 8 * cn],
                        cn * 128, cn * 128, D)
                    gtiles.append(g)
                for w in range(NT):
                    tw = int(TT[w])
                    ps = ps2.tile([128, D], FP, tag="acc", space="PSUM")
                    for t in range(tw):
                        pos = int(off[w]) + t
                        chk, sl = divmod(pos, 8)
                        S = wp.tile([128, 128], BF, tag="S")
                        nc.vector.tensor_tensor(
                            out=S[:], in0=dst_t[:, pos : pos + 1].to_broadcast([128, 128]),
                            in1=iota[:], op=OP.is_equal)
                        nc.tensor.matmul(out=ps[:], lhsT=S[:], rhs=gtiles[chk][:, sl, :],
                                         start=(t == 0), stop=(t == tw - 1))
                    if not hi_pass:
                        nc.vector.tensor_copy(acc16[:, w * D : (w + 1) * D], ps[:])
                    else:
                        bt = wp.tile([128, D], FP, tag="bt")
                        nc.sync.dma_start(bt[:], base_h[w * 128 : (w + 1) * 128, :])
                        accf = wp.tile([128, D], FP, tag="accf")
                        nc.vector.tensor_copy(accf[:], acc16[:, w * D : (w + 1) * D])
                        ot = wp.tile([128, D], FP, tag="ot")
                        nc.vector.tensor_add(ot[:], bt[:], ps[:])
                        nc.vector.tensor_add(ot[:], ot[:], accf[:])
                        nc.sync.dma_start(out_h[w * 128 : (w + 1) * 128, :], ot[:])

    nc.compile()
    return nc


def kernel(node_s, node_vec, edge, edge_difference, edge_dis,
           W1, b1, W2, b2, Wf, bf):
    node_s = np.asarray(node_s, np.float32)
    node_vec = np.asarray(node_vec, np.float32)
    edge = np.asarray(edge)
    edge_dis = np.asarray(edge_dis, np.float32)
    W1 = np.asarray(W1, np.float32); b1 = np.asarray(b1, np.float32)
    W2 = np.asarray(W2, np.float32); b2 = np.asarray(b2, np.float32)
    Wf = np.asarray(Wf, np.float32); bf = np.asarray(bf, np.float32)

    in_maps, TA, TB, offLO, offHI, TLO, THI = _host_prep(
        node_s, node_vec, edge, edge_dis)

    nc = _build_program(TA, TB, offLO, offHI, TLO, THI)

    b2cat = np.concatenate([b2[0:128], b2[256:384]])
    consts = {
        "W1b": W1.astype(BF16),
        "W2c": np.concatenate([W2[:, 0:128], W2[:, 256:384]], 1).astype(BF16),
        "Wfc": np.concatenate(
            [np.concatenate([Wf[:, 0:128], bf[None, 0:128]], 0),
             np.concatenate([Wf[:, 256:384], bf[None, 256:384]], 0)], 1).astype(BF16),
        "b1c": b1.reshape(F, 1).copy(),
        "b2bc": np.tile(b2cat, (128, 1)).astype(np.float32),
        "iota": np.tile(np.arange(128), (128, 1)).astype(BF16),
    }
    for m in in_maps:
        m.update(consts)

    from concourse.bass_utils import run_bass_kernel_spmd
    trace = bool(int(os.environ.get("KERNEL_TRACE", "0")))
    res = run_bass_kernel_spmd(nc, in_maps, core_ids=list(range(NC)), trace=trace)
    kernel.last_results = res

    out = np.concatenate(
        [res.results[c]["out"][: min(SLICE, N - c * SLICE)] for c in range(NC)], 0)
    node_s_out = out[:, 0:F].copy()
    node_vec_out = out[:, F:D].reshape(N, F, 3).copy()
    return (node_vec_out, node_s_out)


# revision 7
# speedup vs baseline: 1.0279x; 1.0279x over previous
"""Trainium2 Bass kernel for nn_Message (GNN message passing), 8 NeuronCores.

Math (from the reference):
    q = (rbf @ Wf + bf) * cc * h            per node, [N, 384]
      rbf[n,k] = sin((k+1)*pi*d_n/5)/d_n, cc = cosine cutoff(d),
      h = silu(node_s @ W1 + b1) @ W2 + b2
    payload m[n] = [ q[:,256:384] (128) | (node_vec * q[:,0:128])(384) ]
    out[d] = base[d] + sum_{edges e: dst[e]=d} m[src[e]],  base = [node_s | node_vec]

Distribution (v3):
  - Nodes in 6400-blocks per core.  Phase 1 computes the bf16 payload slice
    node-major: h1T -> silu -> h = aT.T @ W2cat (+ b2 via a K=1 accumulate
    matmul) and F2 = sinT.T @ Wfcat_aug sharing one PSUM bank;
    sin(k*theta)*cc/d via Chebyshev recurrence, bf*cc folded in as a 21st
    rbf row.  node_s ships pre-transposed bf16, node_vec ships bf16.
  - The payload table is AllGathered in NSPAN span-chunks (separate Shared
    tensors, one writer each) pipelined against phase-1 and phase-2.
  - Phase 2 runs NSPAN passes; pass j gathers payload rows whose src falls
    in span j (host-remapped local indices), as a continuous stream of
    1024-idx dma_gathers (SWDGE ring cap).  One-hot matmuls (S built in
    8-tile batches by one DVE is_equal) accumulate per 128-dst window in
    PSUM; intermediate passes park sums in a bf16 SBUF accumulator.
"""
import sys, os
sys.path.insert(0, "/opt/trn_rl_repo")
import numpy as np
import ml_dtypes

BF16 = ml_dtypes.bfloat16

N, F, E = 50000, 128, 800000
CUTOFF = 5.0
NUM_RBF = 20
K1 = NUM_RBF + 1
NC = 8
SLICE = 6400
NPAD = SLICE * NC
NT = SLICE // 128
D = 512

# span split of each core's slice (in nodes); tile-aligned
SPANS = [(0, 1664), (1664, 1536), (3200, 1664), (4864, 1536)]
NSPAN = len(SPANS)


def _remap(n):
    """Global node id -> (span index, row within span table)."""
    n = np.asarray(n, np.int64)
    k = n % SLICE
    r = n // SLICE
    starts = np.array([s for s, _ in SPANS])
    lens = np.array([l for _, l in SPANS])
    j = np.searchsorted(starts, k, "right") - 1
    return j, r * lens[j] + (k - starts[j])


def _pack_idx(idx, ntok):
    a = np.asarray(idx, np.int16).reshape(ntok // 16, 16).T
    return np.tile(a, (8, 1))


def _host_prep(node_s, node_vec, edge, edge_dis):
    dst = edge[:, 0].astype(np.int64)
    src = edge[:, 1].astype(np.int64)
    sj, srow = _remap(src)

    order = np.argsort(dst, kind="stable")
    dst_s = dst[order]
    srow_s = srow[order]
    sj_s = sj[order]
    core_of = dst_s // SLICE
    win_of = (dst_s % SLICE) // 128
    loc_of = dst_s % 128

    gwin = core_of * NT + win_of
    bounds = np.searchsorted(gwin, np.arange(NC * NT + 1))

    counts = np.zeros((NSPAN, NC, NT), np.int64)
    per_win = {}
    for g in range(NC * NT):
        s, e_ = bounds[g], bounds[g + 1]
        c, w = divmod(g, NT)
        for j in range(NSPAN):
            m = sj_s[s:e_] == j
            per_win[(j, c, w)] = (srow_s[s:e_][m], loc_of[s:e_][m])
            counts[j, c, w] = int(m.sum())

    # static tiles per (span, window): max over cores, >= 1
    TT = np.maximum(1, -(-counts.max(1) // 128)).astype(int)      # [NSPAN, NT]
    OFF = np.concatenate([np.zeros((NSPAN, 1), int), np.cumsum(TT, 1)], 1)
    TOT = OFF[:, -1].astype(int)                                   # tiles per span stream

    ns_pad = np.zeros((NPAD, F), np.float32)
    ns_pad[:N] = node_s
    nv_pad = np.zeros((NPAD, F * 3), np.float32)
    nv_pad[:N] = node_vec.reshape(N, F * 3)
    d_pad = np.ones(NPAD, np.float32)
    d_pad[:N] = edge_dis
    base_all = np.concatenate([ns_pad, nv_pad], 1)

    in_maps = []
    for c in range(NC):
        per_core = {}
        for j in range(NSPAN):
            idx = np.zeros((128, TOT[j] * 8), np.int16)
            dstv = np.full((128, TOT[j]), -1.0, BF16)
            for w in range(NT):
                ls, ll = per_win[(j, c, w)]
                nA = TT[j, w] * 128
                la = np.zeros(nA, np.int64); la[: len(ls)] = ls
                idx[:, OFF[j, w] * 8 : OFF[j, w + 1] * 8] = _pack_idx(la, nA)
                dl = np.full(nA, -1.0, np.float32); dl[: len(ll)] = ll
                dstv[:, OFF[j, w] : OFF[j, w + 1]] = dl.reshape(TT[j, w], 128).T.astype(BF16)
            per_core[f"idx{j}"] = idx
            per_core[f"dst{j}"] = dstv
        sl = slice(c * SLICE, (c + 1) * SLICE)
        per_core.update({
            "base": base_all[sl].copy(),
            "nsT": ns_pad[sl].T.astype(BF16).copy(),
            "nvb": nv_pad[sl].astype(BF16).copy(),
            "d_pt": d_pad[sl].reshape(NT, 128).T.copy(),
        })
        in_maps.append(per_core)
    return in_maps, TT, OFF, TOT


def _build_program(TT, OFF, TOT):
    import concourse.bass as bass
    import concourse.bacc as bacc
    import concourse.mybir as mybir
    import concourse.tile as tile
    from concourse import library_config
    from concourse.masks import make_identity

    FP = mybir.dt.float32
    BF = mybir.dt.bfloat16
    AF = mybir.ActivationFunctionType
    OP = mybir.AluOpType

    nc = bacc.Bacc("TRN2", target_bir_lowering=False, debug=False, num_devices=NC)

    base_h = nc.dram_tensor("base", [SLICE, D], FP, kind="ExternalInput")
    nsT_h = nc.dram_tensor("nsT", [128, SLICE], BF, kind="ExternalInput")
    nvb_h = nc.dram_tensor("nvb", [SLICE, F * 3], BF, kind="ExternalInput")
    dpt_h = nc.dram_tensor("d_pt", [128, NT], FP, kind="ExternalInput")
    idx_h = [nc.dram_tensor(f"idx{j}", [128, int(TOT[j]) * 8], mybir.dt.int16,
                            kind="ExternalInput") for j in range(NSPAN)]
    dst_h = [nc.dram_tensor(f"dst{j}", [128, int(TOT[j])], BF,
                            kind="ExternalInput") for j in range(NSPAN)]
    W1_h = nc.dram_tensor("W1b", [F, F], BF, kind="ExternalInput")
    W2c_h = nc.dram_tensor("W2c", [F, 256], BF, kind="ExternalInput")
    Wfc_h = nc.dram_tensor("Wfc", [K1, 256], BF, kind="ExternalInput")
    b1_h = nc.dram_tensor("b1c", [F, 1], FP, kind="ExternalInput")
    b2r_h = nc.dram_tensor("b2r", [1, 256], BF, kind="ExternalInput")
    iota_h = nc.dram_tensor("iota", [128, 128], BF, kind="ExternalInput")
    out_h = nc.dram_tensor("out", [SLICE, D], FP, kind="ExternalOutput")

    with tile.TileContext(nc) as tc:
        nc.gpsimd.load_library(library_config.mlp)
        with (
            tc.tile_pool(name="dram", bufs=1, space="DRAM") as dram,
            tc.tile_pool(name="const", bufs=1) as cp,
            tc.tile_pool(name="work", bufs=2) as wp,
            tc.tile_pool(name="prep", bufs=1) as pp,
            tc.tile_pool(name="ps1", bufs=2, space="PSUM") as ps1,
            tc.tile_pool(name="ps2", bufs=2, space="PSUM") as ps2,
        ):
            # ---- constants ----
            identb = cp.tile([128, 128], BF)
            make_identity(nc, identb[:])
            W1s = cp.tile([F, F], BF); nc.sync.dma_start(W1s[:], W1_h[:])
            W2c = cp.tile([F, 256], BF); nc.sync.dma_start(W2c[:], W2c_h[:])
            Wfc = cp.tile([K1, 256], BF); nc.sync.dma_start(Wfc[:], Wfc_h[:])
            b1s = cp.tile([F, 1], FP); nc.sync.dma_start(b1s[:], b1_h[:])
            b2r = cp.tile([1, 256], BF); nc.sync.dma_start(b2r[:], b2r_h[:])
            onesb = cp.tile([1, 128], BF)
            nc.vector.memset(onesb[:], 1.0)
            iota = cp.tile([128, 128], BF); nc.sync.dma_start(iota[:], iota_h[:])
            halfpi = cp.tile([128, 1], FP)
            nc.vector.memset(halfpi[:], float(np.pi / 2))
            nsT = cp.tile([128, SLICE], BF)
            nc.sync.dma_start(nsT[:], nsT_h[:])

            m_slice = dram.tile([SLICE, D], BF)
            m_tab = [dram.tile([NC * SPANS[j][1], D], BF, addr_space="Shared",
                               name=f"mtab{j}") for j in range(NSPAN)]

            # ---- prepass ----
            d_all = pp.tile([128, NT], FP)
            nc.sync.dma_start(d_all[:], dpt_h[:])
            srec = pp.tile([128, K1, NT], FP)
            sh = pp.tile([128, NT], FP)
            ch = pp.tile([128, NT], FP)
            nc.scalar.activation(sh[:], d_all[:], AF.Sin, scale=float(np.pi / 10))
            nc.scalar.activation(ch[:], d_all[:], AF.Sin, bias=halfpi[:], scale=float(-np.pi / 10))
            s1 = srec[:, 0, :]
            nc.vector.tensor_mul(s1, sh[:], ch[:])
            nc.vector.tensor_scalar_mul(s1, s1, 2.0)
            cth = pp.tile([128, NT], FP)
            nc.vector.tensor_mul(cth[:], sh[:], sh[:])
            nc.vector.tensor_scalar(cth[:], cth[:], -2.0, 1.0, OP.mult, OP.add)
            c2 = pp.tile([128, NT], FP)
            nc.vector.tensor_scalar_mul(c2[:], cth[:], 2.0)
            nc.vector.tensor_mul(srec[:, 1, :], c2[:], s1)
            for k in range(2, NUM_RBF):
                nc.vector.tensor_mul(srec[:, k, :], c2[:], srec[:, k - 1, :])
                nc.vector.tensor_sub(srec[:, k, :], srec[:, k, :], srec[:, k - 2, :])
            mask = pp.tile([128, NT], FP)
            nc.vector.tensor_scalar(mask[:], d_all[:], CUTOFF, None, OP.is_le)
            cc = srec[:, NUM_RBF, :]
            nc.vector.tensor_scalar(cc, cth[:], 0.5, 0.5, OP.mult, OP.add)
            nc.vector.tensor_mul(cc, cc, mask[:])
            sall = pp.tile([128, NT], FP)
            nc.vector.reciprocal(sall[:], d_all[:])
            nc.vector.tensor_mul(sall[:], sall[:], cc)
            nc.vector.tensor_tensor(
                out=srec[:, 0:NUM_RBF, :], in0=srec[:, 0:NUM_RBF, :],
                in1=sall[:, None, :].to_broadcast([128, NUM_RBF, NT]), op=OP.mult)
            srecb = pp.tile([128, K1, NT], BF)
            nc.vector.tensor_copy(srecb[:], srec[:])

            # ---- phase 1 + span AGs ----
            for j, (sp0, spl) in enumerate(SPANS):
                t0j, t1j = sp0 // 128, (sp0 + spl) // 128
                for t in range(t0j, t1j):
                    r0, r1 = t * 128, (t + 1) * 128
                    h1_ps = ps1.tile([128, F], FP, tag="h1", space="PSUM")
                    nc.tensor.matmul(out=h1_ps[:], lhsT=W1s[:], rhs=nsT[:, r0:r1],
                                     start=True, stop=True)
                    aT = wp.tile([128, F], BF, tag="aT")
                    nc.scalar.activation(aT[:], h1_ps[:], AF.Silu, bias=b1s[:])
                    sinT_ps = ps1.tile([K1, 128], BF, tag="st", space="PSUM")
                    nc.tensor.transpose(out=sinT_ps[:], in_=srecb[:, :, t], identity=identb[:])
                    sinT = wp.tile([K1, 128], BF, tag="stb")
                    nc.vector.tensor_copy(out=sinT[:], in_=sinT_ps[:])
                    hf_ps = ps1.tile([128, 512], FP, tag="hf", space="PSUM")
                    nc.tensor.matmul(out=hf_ps[:, 0:256], lhsT=aT[:], rhs=W2c[:],
                                     start=True, stop=False)
                    nc.tensor.matmul(out=hf_ps[:, 0:256], lhsT=onesb[:], rhs=b2r[:],
                                     start=False, stop=True)
                    nc.tensor.matmul(out=hf_ps[:, 256:512], lhsT=sinT[:], rhs=Wfc[:],
                                     start=True, stop=True)
                    hc = wp.tile([128, 256], FP, tag="hc")
                    nc.vector.tensor_copy(hc[:], hf_ps[:, 0:256])
                    q = wp.tile([128, 256], BF, tag="q")
                    nc.vector.tensor_mul(q[:], hf_ps[:, 256:512], hc[:])
                    pay = wp.tile([128, D], BF, tag="pay")
                    nc.vector.tensor_copy(pay[:, 0:128], q[:, 128:256])
                    NV = wp.tile([128, F * 3], BF, tag="NV")
                    nc.sync.dma_start(NV[:], nvb_h[r0:r1, :])
                    nc.vector.tensor_tensor(
                        out=pay[:, 128:512].rearrange("p (f c) -> p f c", c=3),
                        in0=NV[:].rearrange("p (f c) -> p f c", c=3),
                        in1=q[:, 0:128, None].to_broadcast([128, F, 3]),
                        op=OP.mult)
                    nc.sync.dma_start(m_slice[r0:r1, :], pay[:])
                nc.gpsimd.collective_compute(
                    "AllGather", OP.bypass,
                    replica_groups=[list(range(NC))],
                    ins=[m_slice[sp0 : sp0 + spl, :].opt()],
                    outs=[m_tab[j][:].opt()],
                )

            # ---- phase 2: NSPAN gather/matmul passes ----
            acc16 = pp.tile([128, NT * D], BF)
            for j in range(NSPAN):
                ntiles = int(TOT[j])
                idx_t = pp.tile([128, ntiles * 8], mybir.dt.int16, name=f"idxsb{j}")
                nc.sync.dma_start(idx_t[:], idx_h[j][:])
                dst_t = pp.tile([128, ntiles], BF, name=f"dstsb{j}")
                nc.sync.dma_start(dst_t[:], dst_h[j][:])
                nchunks = -(-ntiles // 8)
                gtiles, stiles = [], []
                for c in range(nchunks):
                    cn = min(8, ntiles - c * 8)
                    g = wp.tile([128, 8, D], BF, tag="g", bufs=6)
                    nc.gpsimd.dma_gather(
                        g[:, 0:cn, :], m_tab[j][:], idx_t[:, c * 64 : c * 64 + 8 * cn],
                        cn * 128, cn * 128, D)
                    gtiles.append(g)
                    S = wp.tile([128, 8, 128], BF, tag="S", bufs=6)
                    nc.vector.tensor_tensor(
                        out=S[:, 0:cn, :],
                        in0=dst_t[:, c * 8 : c * 8 + cn, None].to_broadcast([128, cn, 128]),
                        in1=iota[:, None, :].to_broadcast([128, cn, 128]),
                        op=OP.is_equal)
                    stiles.append(S)
                for w in range(NT):
                    tw = int(TT[j, w])
                    ps = ps2.tile([128, D], FP, tag="acc", space="PSUM")
                    for t in range(tw):
                        pos = int(OFF[j, w]) + t
                        chk, sl = divmod(pos, 8)
                        nc.tensor.matmul(out=ps[:], lhsT=stiles[chk][:, sl, :],
                                         rhs=gtiles[chk][:, sl, :],
                                         start=(t == 0), stop=(t == tw - 1))
                    wsl = slice(w * D, (w + 1) * D)
                    if j == 0:
                        nc.vector.tensor_copy(acc16[:, wsl], ps[:])
                    elif j < NSPAN - 1:
                        accf = wp.tile([128, D], FP, tag="accf")
                        nc.vector.tensor_copy(accf[:], acc16[:, wsl])
                        nc.vector.tensor_add(acc16[:, wsl], ps[:], accf[:])
                    else:
                        bt = wp.tile([128, D], FP, tag="bt")
                        nc.sync.dma_start(bt[:], base_h[w * 128 : (w + 1) * 128, :])
                        accf = wp.tile([128, D], FP, tag="accf")
                        nc.vector.tensor_copy(accf[:], acc16[:, wsl])
                        ot = wp.tile([128, D], FP, tag="ot")
                        nc.vector.tensor_add(ot[:], bt[:], ps[:])
                        nc.vector.tensor_add(ot[:], ot[:], accf[:])
                        nc.sync.dma_start(out_h[w * 128 : (w + 1) * 128, :], ot[:])

    nc.compile()
    return nc


def kernel(node_s, node_vec, edge, edge_difference, edge_dis,
           W1, b1, W2, b2, Wf, bf):
    node_s = np.asarray(node_s, np.float32)
    node_vec = np.asarray(node_vec, np.float32)
    edge = np.asarray(edge)
    edge_dis = np.asarray(edge_dis, np.float32)
    W1 = np.asarray(W1, np.float32); b1 = np.asarray(b1, np.float32)
    W2 = np.asarray(W2, np.float32); b2 = np.asarray(b2, np.float32)
    Wf = np.asarray(Wf, np.float32); bf = np.asarray(bf, np.float32)

    in_maps, TT, OFF, TOT = _host_prep(node_s, node_vec, edge, edge_dis)
    nc = _build_program(TT, OFF, TOT)

    consts = {
        "W1b": W1.astype(BF16),
        "W2c": np.concatenate([W2[:, 0:128], W2[:, 256:384]], 1).astype(BF16),
        "Wfc": np.concatenate(
            [np.concatenate([Wf[:, 0:128], bf[None, 0:128]], 0),
             np.concatenate([Wf[:, 256:384], bf[None, 256:384]], 0)], 1).astype(BF16),
        "b1c": b1.reshape(F, 1).copy(),
        "b2r": np.concatenate([b2[0:128], b2[256:384]]).reshape(1, 256).astype(BF16),
        "iota": np.tile(np.arange(128), (128, 1)).astype(BF16),
    }
    for m in in_maps:
        m.update(consts)

    from concourse.bass_utils import run_bass_kernel_spmd
    trace = bool(int(os.environ.get("KERNEL_TRACE", "0")))
    res = run_bass_kernel_spmd(nc, in_maps, core_ids=list(range(NC)), trace=trace)
    kernel.last_results = res

    out = np.concatenate(
        [res.results[c]["out"][: min(SLICE, N - c * SLICE)] for c in range(NC)], 0)
    node_s_out = out[:, 0:F].copy()
    node_vec_out = out[:, F:D].reshape(N, F, 3).copy()
    return (node_vec_out, node_s_out)


# revision 8
# speedup vs baseline: 1.0423x; 1.0140x over previous
"""Trainium2 Bass kernel for nn_Message (GNN message passing), 8 NeuronCores.

Math (from the reference):
    q = (rbf @ Wf + bf) * cc * h            per node, [N, 384]
      rbf[n,k] = sin((k+1)*pi*d_n/5)/d_n, cc = cosine cutoff(d),
      h = silu(node_s @ W1 + b1) @ W2 + b2
    payload m[n] = [ q[:,256:384] (128) | (node_vec * q[:,0:128])(384) ]
    out[d] = base[d] + sum_{edges e: dst[e]=d} m[src[e]],  base = [node_s | node_vec]

Distribution (v3):
  - Nodes in 6400-blocks per core.  Phase 1 computes the bf16 payload slice
    node-major: h1T -> silu -> h = aT.T @ W2cat (+ b2 via a K=1 accumulate
    matmul) and F2 = sinT.T @ Wfcat_aug sharing one PSUM bank;
    sin(k*theta)*cc/d via Chebyshev recurrence, bf*cc folded in as a 21st
    rbf row.  node_s ships pre-transposed bf16, node_vec ships bf16.
  - The payload table is AllGathered in NSPAN span-chunks (separate Shared
    tensors, one writer each) pipelined against phase-1 and phase-2.
  - Phase 2 runs NSPAN passes; pass j gathers payload rows whose src falls
    in span j (host-remapped local indices), as a continuous stream of
    1024-idx dma_gathers (SWDGE ring cap).  One-hot matmuls (S built in
    8-tile batches by one DVE is_equal) accumulate per 128-dst window in
    PSUM; intermediate passes park sums in a bf16 SBUF accumulator.
"""
import sys, os
sys.path.insert(0, "/opt/trn_rl_repo")
import numpy as np
import ml_dtypes

BF16 = ml_dtypes.bfloat16

N, F, E = 50000, 128, 800000
CUTOFF = 5.0
NUM_RBF = 20
K1 = NUM_RBF + 1
NC = 8
SLICE = 6400
NPAD = SLICE * NC
NT = SLICE // 128
D = 512

# span split of each core's slice (in nodes); tile-aligned
SPANS = [(0, 1664), (1664, 1536), (3200, 1664), (4864, 1536)]
NSPAN = len(SPANS)


def _remap(n):
    """Global node id -> (span index, row within span table)."""
    n = np.asarray(n, np.int64)
    k = n % SLICE
    r = n // SLICE
    starts = np.array([s for s, _ in SPANS])
    lens = np.array([l for _, l in SPANS])
    j = np.searchsorted(starts, k, "right") - 1
    return j, r * lens[j] + (k - starts[j])


def _pack_idx(idx, ntok):
    a = np.asarray(idx, np.int16).reshape(ntok // 16, 16).T
    return np.tile(a, (8, 1))


def _host_prep(node_s, node_vec, edge, edge_dis):
    dst = edge[:, 0].astype(np.int64)
    src = edge[:, 1].astype(np.int64)
    sj, srow = _remap(src)

    order = np.argsort(dst, kind="stable")
    dst_s = dst[order]
    srow_s = srow[order]
    sj_s = sj[order]
    core_of = dst_s // SLICE
    win_of = (dst_s % SLICE) // 128
    loc_of = dst_s % 128

    gwin = core_of * NT + win_of
    bounds = np.searchsorted(gwin, np.arange(NC * NT + 1))

    counts = np.zeros((NSPAN, NC, NT), np.int64)
    per_win = {}
    for g in range(NC * NT):
        s, e_ = bounds[g], bounds[g + 1]
        c, w = divmod(g, NT)
        for j in range(NSPAN):
            m = sj_s[s:e_] == j
            per_win[(j, c, w)] = (srow_s[s:e_][m], loc_of[s:e_][m])
            counts[j, c, w] = int(m.sum())

    # static tiles per (span, window): max over cores, >= 1
    TT = np.maximum(1, -(-counts.max(1) // 128)).astype(int)      # [NSPAN, NT]
    OFF = np.concatenate([np.zeros((NSPAN, 1), int), np.cumsum(TT, 1)], 1)
    TOT = OFF[:, -1].astype(int)                                   # tiles per span stream

    ns_pad = np.zeros((NPAD, F), np.float32)
    ns_pad[:N] = node_s
    nv_pad = np.zeros((NPAD, F * 3), np.float32)
    nv_pad[:N] = node_vec.reshape(N, F * 3)
    d_pad = np.ones(NPAD, np.float32)
    d_pad[:N] = edge_dis
    base_all = np.concatenate([ns_pad, nv_pad], 1)

    in_maps = []
    for c in range(NC):
        per_core = {}
        for j in range(NSPAN):
            idx = np.zeros((128, TOT[j] * 8), np.int16)
            dstv = np.full((128, TOT[j]), -1.0, BF16)
            for w in range(NT):
                ls, ll = per_win[(j, c, w)]
                nA = TT[j, w] * 128
                la = np.zeros(nA, np.int64); la[: len(ls)] = ls
                idx[:, OFF[j, w] * 8 : OFF[j, w + 1] * 8] = _pack_idx(la, nA)
                dl = np.full(nA, -1.0, np.float32); dl[: len(ll)] = ll
                dstv[:, OFF[j, w] : OFF[j, w + 1]] = dl.reshape(TT[j, w], 128).T.astype(BF16)
            per_core[f"idx{j}"] = idx
            per_core[f"dst{j}"] = dstv
        sl = slice(c * SLICE, (c + 1) * SLICE)
        per_core.update({
            "base": base_all[sl].copy(),
            "nsT": ns_pad[sl].T.astype(BF16).copy(),
            "nvb": nv_pad[sl].astype(BF16).copy(),
            "d_pt": d_pad[sl].reshape(NT, 128).T.copy(),
        })
        in_maps.append(per_core)
    return in_maps, TT, OFF, TOT


def _build_program(TT, OFF, TOT):
    import concourse.bass as bass
    import concourse.bacc as bacc
    import concourse.mybir as mybir
    import concourse.tile as tile
    from concourse import library_config
    from concourse.masks import make_identity

    FP = mybir.dt.float32
    BF = mybir.dt.bfloat16
    AF = mybir.ActivationFunctionType
    OP = mybir.AluOpType

    nc = bacc.Bacc("TRN2", target_bir_lowering=False, debug=False, num_devices=NC)

    base_h = nc.dram_tensor("base", [SLICE, D], FP, kind="ExternalInput")
    nsT_h = nc.dram_tensor("nsT", [128, SLICE], BF, kind="ExternalInput")
    nvb_h = nc.dram_tensor("nvb", [SLICE, F * 3], BF, kind="ExternalInput")
    dpt_h = nc.dram_tensor("d_pt", [128, NT], FP, kind="ExternalInput")
    idx_h = [nc.dram_tensor(f"idx{j}", [128, int(TOT[j]) * 8], mybir.dt.int16,
                            kind="ExternalInput") for j in range(NSPAN)]
    dst_h = [nc.dram_tensor(f"dst{j}", [128, int(TOT[j])], BF,
                            kind="ExternalInput") for j in range(NSPAN)]
    W1_h = nc.dram_tensor("W1b", [F, F], BF, kind="ExternalInput")
    W2c_h = nc.dram_tensor("W2c", [F, 256], BF, kind="ExternalInput")
    Wfc_h = nc.dram_tensor("Wfc", [K1, 256], BF, kind="ExternalInput")
    b1_h = nc.dram_tensor("b1c", [F, 1], FP, kind="ExternalInput")
    b2r_h = nc.dram_tensor("b2r", [1, 256], BF, kind="ExternalInput")
    iota_h = nc.dram_tensor("iota", [128, 128], BF, kind="ExternalInput")
    out_h = nc.dram_tensor("out", [SLICE, D], FP, kind="ExternalOutput")

    with tile.TileContext(nc) as tc:
        nc.gpsimd.load_library(library_config.mlp)
        with (
            tc.tile_pool(name="dram", bufs=1, space="DRAM") as dram,
            tc.tile_pool(name="const", bufs=1) as cp,
            tc.tile_pool(name="work", bufs=2) as wp,
            tc.tile_pool(name="prep", bufs=1) as pp,
            tc.tile_pool(name="ps1", bufs=2, space="PSUM") as ps1,
            tc.tile_pool(name="ps2", bufs=2, space="PSUM") as ps2,
        ):
            # ---- constants ----
            identb = cp.tile([128, 128], BF)
            make_identity(nc, identb[:])
            W1s = cp.tile([F, F], BF); nc.sync.dma_start(W1s[:], W1_h[:])
            W2c = cp.tile([F, 256], BF); nc.sync.dma_start(W2c[:], W2c_h[:])
            Wfc = cp.tile([K1, 256], BF); nc.sync.dma_start(Wfc[:], Wfc_h[:])
            b1s = cp.tile([F, 1], FP); nc.sync.dma_start(b1s[:], b1_h[:])
            b2r = cp.tile([1, 256], BF); nc.sync.dma_start(b2r[:], b2r_h[:])
            onesb = cp.tile([1, 128], BF)
            nc.vector.memset(onesb[:], 1.0)
            iota = cp.tile([128, 128], BF); nc.sync.dma_start(iota[:], iota_h[:])
            halfpi = cp.tile([128, 1], FP)
            nc.vector.memset(halfpi[:], float(np.pi / 2))
            nsT = cp.tile([128, SLICE], BF)
            nc.sync.dma_start(nsT[:], nsT_h[:])

            m_slice = dram.tile([SLICE, D], BF)
            m_tab = [dram.tile([NC * SPANS[j][1], D], BF, addr_space="Shared",
                               name=f"mtab{j}") for j in range(NSPAN)]

            # ---- prepass ----
            d_all = pp.tile([128, NT], FP)
            nc.sync.dma_start(d_all[:], dpt_h[:])
            srec = pp.tile([128, K1, NT], FP)
            sh = pp.tile([128, NT], FP)
            ch = pp.tile([128, NT], FP)
            nc.scalar.activation(sh[:], d_all[:], AF.Sin, scale=float(np.pi / 10))
            nc.scalar.activation(ch[:], d_all[:], AF.Sin, bias=halfpi[:], scale=float(-np.pi / 10))
            s1 = srec[:, 0, :]
            nc.vector.tensor_mul(s1, sh[:], ch[:])
            nc.vector.tensor_scalar_mul(s1, s1, 2.0)
            cth = pp.tile([128, NT], FP)
            nc.vector.tensor_mul(cth[:], sh[:], sh[:])
            nc.vector.tensor_scalar(cth[:], cth[:], -2.0, 1.0, OP.mult, OP.add)
            c2 = pp.tile([128, NT], FP)
            nc.vector.tensor_scalar_mul(c2[:], cth[:], 2.0)
            nc.vector.tensor_mul(srec[:, 1, :], c2[:], s1)
            for k in range(2, NUM_RBF):
                nc.vector.tensor_mul(srec[:, k, :], c2[:], srec[:, k - 1, :])
                nc.vector.tensor_sub(srec[:, k, :], srec[:, k, :], srec[:, k - 2, :])
            mask = pp.tile([128, NT], FP)
            nc.vector.tensor_scalar(mask[:], d_all[:], CUTOFF, None, OP.is_le)
            cc = srec[:, NUM_RBF, :]
            nc.vector.tensor_scalar(cc, cth[:], 0.5, 0.5, OP.mult, OP.add)
            nc.vector.tensor_mul(cc, cc, mask[:])
            sall = pp.tile([128, NT], FP)
            nc.vector.reciprocal(sall[:], d_all[:])
            nc.vector.tensor_mul(sall[:], sall[:], cc)
            nc.vector.tensor_tensor(
                out=srec[:, 0:NUM_RBF, :], in0=srec[:, 0:NUM_RBF, :],
                in1=sall[:, None, :].to_broadcast([128, NUM_RBF, NT]), op=OP.mult)
            srecb = pp.tile([128, K1, NT], BF)
            nc.vector.tensor_copy(srecb[:], srec[:])

            # ---- phase 1 + span AGs ----
            for j, (sp0, spl) in enumerate(SPANS):
                t0j, t1j = sp0 // 128, (sp0 + spl) // 128
                for t in range(t0j, t1j):
                    r0, r1 = t * 128, (t + 1) * 128
                    h1_ps = ps1.tile([128, F], FP, tag="h1", space="PSUM", bufs=1)
                    nc.tensor.matmul(out=h1_ps[:], lhsT=W1s[:], rhs=nsT[:, r0:r1],
                                     start=True, stop=True)
                    aT = wp.tile([128, F], BF, tag="aT")
                    nc.scalar.activation(aT[:], h1_ps[:], AF.Silu, bias=b1s[:])
                    sinT_ps = ps1.tile([K1, 128], BF, tag="st", space="PSUM", bufs=1)
                    nc.tensor.transpose(out=sinT_ps[:], in_=srecb[:, :, t], identity=identb[:])
                    sinT = wp.tile([K1, 128], BF, tag="stb")
                    nc.vector.tensor_copy(out=sinT[:], in_=sinT_ps[:])
                    h_ps = ps1.tile([128, 256], FP, tag="h2", space="PSUM")
                    nc.tensor.matmul(out=h_ps[:, 0:256], lhsT=aT[:], rhs=W2c[:],
                                     start=True, stop=False)
                    nc.tensor.matmul(out=h_ps[:, 0:256], lhsT=onesb[:], rhs=b2r[:],
                                     start=False, stop=True)
                    F2_ps = ps1.tile([128, 256], FP, tag="f2", space="PSUM")
                    nc.tensor.matmul(out=F2_ps[:], lhsT=sinT[:], rhs=Wfc[:],
                                     start=True, stop=True)
                    hc = wp.tile([128, 256], FP, tag="hc")
                    nc.vector.tensor_copy(hc[:], h_ps[:])
                    q = wp.tile([128, 256], BF, tag="q")
                    nc.vector.tensor_mul(q[:], F2_ps[:], hc[:])
                    pay = wp.tile([128, D], BF, tag="pay")
                    nc.vector.tensor_copy(pay[:, 0:128], q[:, 128:256])
                    NV = wp.tile([128, F * 3], BF, tag="NV")
                    nc.sync.dma_start(NV[:], nvb_h[r0:r1, :])
                    nc.vector.tensor_tensor(
                        out=pay[:, 128:512].rearrange("p (f c) -> p f c", c=3),
                        in0=NV[:].rearrange("p (f c) -> p f c", c=3),
                        in1=q[:, 0:128, None].to_broadcast([128, F, 3]),
                        op=OP.mult)
                    nc.sync.dma_start(m_slice[r0:r1, :], pay[:])
                nc.gpsimd.collective_compute(
                    "AllGather", OP.bypass,
                    replica_groups=[list(range(NC))],
                    ins=[m_slice[sp0 : sp0 + spl, :].opt()],
                    outs=[m_tab[j][:].opt()],
                )

            # ---- phase 2: NSPAN gather/matmul passes, position-order emission ----
            acc16 = pp.tile([128, NT * D], BF)
            for j in range(NSPAN):
                ntiles = int(TOT[j])
                idx_t = pp.tile([128, ntiles * 8], mybir.dt.int16, name=f"idxsb{j}")
                nc.sync.dma_start(idx_t[:], idx_h[j][:])
                dst_t = pp.tile([128, ntiles], BF, name=f"dstsb{j}")
                nc.sync.dma_start(dst_t[:], dst_h[j][:])
                g = S = ps = None
                w = 0
                for pos in range(ntiles):
                    if pos % 8 == 0:
                        cn = min(8, ntiles - pos)
                        g = wp.tile([128, 8, D], BF, tag="g", bufs=6)
                        nc.gpsimd.dma_gather(
                            g[:, 0:cn, :], m_tab[j][:],
                            idx_t[:, pos * 8 : (pos + cn) * 8],
                            cn * 128, cn * 128, D)
                        S = wp.tile([128, 8, 128], BF, tag="S", bufs=6)
                        nc.vector.tensor_tensor(
                            out=S[:, 0:cn, :],
                            in0=dst_t[:, pos : pos + cn, None].to_broadcast([128, cn, 128]),
                            in1=iota[:, None, :].to_broadcast([128, cn, 128]),
                            op=OP.is_equal)
                    sl = pos % 8
                    if pos == int(OFF[j, w]):
                        ps = ps2.tile([128, D], FP, tag="acc", space="PSUM")
                    last = pos == int(OFF[j, w + 1]) - 1
                    nc.tensor.matmul(out=ps[:], lhsT=S[:, sl, :], rhs=g[:, sl, :],
                                     start=(pos == int(OFF[j, w])), stop=last)
                    if last:
                        wsl = slice(w * D, (w + 1) * D)
                        if j == 0:
                            nc.vector.tensor_copy(acc16[:, wsl], ps[:])
                        elif j < NSPAN - 1:
                            accf = wp.tile([128, D], FP, tag="accf")
                            nc.vector.tensor_copy(accf[:], acc16[:, wsl])
                            nc.vector.tensor_add(acc16[:, wsl], ps[:], accf[:])
                        else:
                            bt = wp.tile([128, D], FP, tag="bt")
                            nc.sync.dma_start(bt[:], base_h[w * 128 : (w + 1) * 128, :])
                            accf = wp.tile([128, D], FP, tag="accf")
                            nc.vector.tensor_copy(accf[:], acc16[:, wsl])
                            ot = wp.tile([128, D], FP, tag="ot")
                            nc.vector.tensor_add(ot[:], bt[:], ps[:])
                            nc.vector.tensor_add(ot[:], ot[:], accf[:])
                            nc.sync.dma_start(out_h[w * 128 : (w + 1) * 128, :], ot[:])
                        w += 1

    nc.compile()
    return nc


def kernel(node_s, node_vec, edge, edge_difference, edge_dis,
           W1, b1, W2, b2, Wf, bf):
    node_s = np.asarray(node_s, np.float32)
    node_vec = np.asarray(node_vec, np.float32)
    edge = np.asarray(edge)
    edge_dis = np.asarray(edge_dis, np.float32)
    W1 = np.asarray(W1, np.float32); b1 = np.asarray(b1, np.float32)
    W2 = np.asarray(W2, np.float32); b2 = np.asarray(b2, np.float32)
    Wf = np.asarray(Wf, np.float32); bf = np.asarray(bf, np.float32)

    in_maps, TT, OFF, TOT = _host_prep(node_s, node_vec, edge, edge_dis)
    nc = _build_program(TT, OFF, TOT)

    consts = {
        "W1b": W1.astype(BF16),
        "W2c": np.concatenate([W2[:, 0:128], W2[:, 256:384]], 1).astype(BF16),
        "Wfc": np.concatenate(
            [np.concatenate([Wf[:, 0:128], bf[None, 0:128]], 0),
             np.concatenate([Wf[:, 256:384], bf[None, 256:384]], 0)], 1).astype(BF16),
        "b1c": b1.reshape(F, 1).copy(),
        "b2r": np.concatenate([b2[0:128], b2[256:384]]).reshape(1, 256).astype(BF16),
        "iota": np.tile(np.arange(128), (128, 1)).astype(BF16),
    }
    for m in in_maps:
        m.update(consts)

    from concourse.bass_utils import run_bass_kernel_spmd
    trace = bool(int(os.environ.get("KERNEL_TRACE", "0")))
    res = run_bass_kernel_spmd(nc, in_maps, core_ids=list(range(NC)), trace=trace)
    kernel.last_results = res

    out = np.concatenate(
        [res.results[c]["out"][: min(SLICE, N - c * SLICE)] for c in range(NC)], 0)
    node_s_out = out[:, 0:F].copy()
    node_vec_out = out[:, F:D].reshape(N, F, 3).copy()
    return (node_vec_out, node_s_out)
